# revision 33
# baseline (speedup 1.0000x reference)
"""Trainium2 Bass kernel for AttentionWithCAE.

Reference computation (B=8, N=1024, C=768, H=12, hd=64):
    qkv  = x @ qkv_w.T + concat(q_bias, 0, v_bias)
    q,k,v per head; attn = softmax(mask(q*scale @ k.T)); out = attn @ v
    final = out @ proj_w.T + proj_b

Sharding: pure data parallel -- batch b on core b, weights replicated,
no collectives.

Structural ideas (in rough order of impact):

1. Key gather: the mask removes ~50% of the KEYS (True = masked out;
   queries are never masked).  The host gathers only the unmasked key
   tokens per batch and pads to NK=640 (max real count is 530), so
   scores, exp, attn@v and the k/v projections shrink by 0.625.  Exact:
   softmax is permutation-invariant over keys; padding keys have zeroed
   x columns and a -30000 exp bias so they contribute exactly 0.

2. Scores transposed [k, q] with the two heads of a pair side by side in
   one [128, 1024] PSUM tile per q-chunk (tileA = q0:512, tileB =
   q512:1024; head hi in columns hi*512..).  The two K=64 matmuls
   filling a tile target row groups 0:64 / 64:128 and are emitted
   back-to-back, so they run CONCURRENTLY on the PE array (measured
   dt ~5ns).  One exp ACTIVATE per tile covers both heads (mask bias is
   per-partition = per key, identical for both).

3. ACT (exp) paces the attention phases.  Pair p's attn@v (with the
   baked ones-column giving the softmax denominators as PSUM row 64) is
   DEFERRED into phase p+1 and interleaved kt-by-kt so its matmuls never
   wait on fresh exps; the last AV chunk is pulled one slot early
   (slot 3) so the normalization chain (slot 4) completes by the phase
   boundary and the next pair's AV never stalls on the PSUM banks.

4. Normalization: denominator row -> partition 0 (single-partition
   custom/ISA ops only work at base 0), reciprocal_approx_fast on DVE,
   partition_broadcast on the otherwise-idle GPSIMD, then one DVE
   multiply per (head, q-chunk) STRAIGHT from the AV PSUM into aoT.

5. DMA: descriptor rows stripe across all 16 DMA engines, so few FAT
   DMAs beat many small ones; dispatch costs ~0.7us per dma_start on an
   engine queue, so inputs are host-packed into 8 large tensors (rows
   1.5-12KB), issued round-robin over the sync/scalar/gpsimd queues in
   earliest-need order.  Output is bf16 (host upcasts).

6. HAM warm-up: ~4us of F=512 garbage matmuls during the DMA dead time
   so the PE is at full clock when real work arrives (F=8 matmuls do
   NOT warm it -- only ~5% array duty).

7. Tail: proj is ot-major from the scores PSUM slots; ot0/ot1's c<5
   matmuls are emitted before the c==5 ones so the last pair's
   normalization latency is fully hidden behind real work.
"""

import sys

sys.path.insert(0, "/opt/trn_rl_repo")

from contextlib import ExitStack

import numpy as np
import ml_dtypes

import concourse.bass as bass
import concourse.bacc as bacc
import concourse.mybir as mybir
from concourse import tile
from concourse.bass_utils import run_bass_kernel_spmd

B, N, C = 8, 1024, 768
H, HD = 12, 64
F3 = 3 * C  # 2304
SCALE = HD ** -0.5
F32 = mybir.dt.float32
BF16 = mybir.dt.bfloat16
Act = mybir.ActivationFunctionType

MASK_NEG = -30000.0

NK = 640  # padded unmasked-key count (max real count is ~530)
NKT = NK // 128  # 5 key tiles
CT = C // 128  # 6 contraction tiles
NPAIR = H // 2  # 6 head pairs

_CACHE = {}


def _build_nc():
    nc = bacc.Bacc(None, target_bir_lowering=False)

    # host-packed layouts (see _pack_weights / kernel()):
    #   xTr  [128, CT*1024] queries, c-major:   [p, c*1024 + t]
    #   xkr  [128, CT*640]  gathered keys, c-major
    #   wqk0 [128, 1536]    q/k weights feature-tile 0: [wq_f0 | wk_f0]
    #   wqf  [128, CT*768]  q weights, feature-tile-major
    #   wkf  [128, CT*768]  k weights, feature-tile-major
    #   wvc  [128, CT*768]  v weights, c-major
    #   pwf  [128, CT*768]  proj weights, out-tile-major
    xT_d = nc.declare_dram_parameter("xTr", [128, CT * 1024], BF16, isOutput=False)
    xk_d = nc.declare_dram_parameter("xkr", [128, CT * 640], BF16, isOutput=False)
    wqk0_d = nc.declare_dram_parameter("wqk0", [128, 1536], BF16, isOutput=False)
    wqf_d = nc.declare_dram_parameter("wqf", [128, CT * 768], BF16, isOutput=False)
    wkf_d = nc.declare_dram_parameter("wkf", [128, CT * 768], BF16, isOutput=False)
    wvc_d = nc.declare_dram_parameter("wvc", [128, CT * 768], BF16, isOutput=False)
    pwf_d = nc.declare_dram_parameter("pwf", [128, CT * 768], BF16, isOutput=False)
    qb_d = nc.declare_dram_parameter("qb", [C], F32, isOutput=False)
    mb_d = nc.declare_dram_parameter("mb", [NK], F32, isOutput=False)
    pb_d = nc.declare_dram_parameter("pb", [C], F32, isOutput=False)
    out_d = nc.declare_dram_parameter("out", [C, N], BF16, isOutput=True)

    with ExitStack() as ctx:
        tc = ctx.enter_context(tile.TileContext(nc))
        pool = ctx.enter_context(tc.tile_pool(name="main", bufs=1))
        psum = ctx.enter_context(tc.tile_pool(name="psum", bufs=1, space="PSUM"))

        # preload the exp table set while DMAs run (no data dependency)
        tiny = pool.tile([1, 8], F32)
        nc.vector.memset(tiny, 0.0)
        tiny2 = pool.tile([1, 8], F32)
        nc.scalar.activation(tiny2, tiny, Act.Exp)

        qb_sb = pool.tile([128, CT], F32)
        nc.sync.dma_start(out=qb_sb, in_=qb_d.rearrange("(i p) -> p i", p=128))
        mb_sb = pool.tile([128, NKT], F32)
        nc.scalar.dma_start(out=mb_sb, in_=mb_d.rearrange("(i p) -> p i", p=128))
        pb_sb = pool.tile([128, CT], F32)
        nc.gpsimd.dma_start(out=pb_sb, in_=pb_d.rearrange("(i p) -> p i", p=128))

        # HAM warm-up: full-width F=512 matmuls (high array duty) bridging
        # the whole DMA wait so real work starts at 2.4GHz.
        warm_w = pool.tile([128, 128], BF16, name="warmw")
        nc.vector.memset(warm_w, 0.25)
        warm_x = pool.tile([128, 512], BF16, name="warmx")
        nc.vector.memset(warm_x, 0.25)
        wps = psum.tile([128, 512], F32, tag="psAV00", bufs=1, name="warm")
        for wi in range(80):
            nc.tensor.matmul(wps, lhsT=warm_w, rhs=warm_x, start=True, stop=True)

        xTall = pool.tile([128, CT * 1024], BF16, name="xTall")
        xkall = pool.tile([128, CT * 640], BF16, name="xkall")
        wqk0 = pool.tile([128, 1536], BF16, name="wqk0")
        wvc = pool.tile([128, CT * 768], BF16, name="wvc")
        wqf = pool.tile([128, CT * 768], BF16, name="wqf")
        wkf = pool.tile([128, CT * 768], BF16, name="wkf")
        pwf = pool.tile([128, CT * 768], BF16, name="pwf")

        def xTs(c):
            return xTall[:, c * 1024 : (c + 1) * 1024]

        def xks(c):
            return xkall[:, c * 640 : (c + 1) * 640]

        # input DMAs: few fat transfers, earliest-need first, spread over
        # three dispatch queues.
        nc.sync.dma_start(out=wqk0, in_=wqk0_d[:, :])
        nc.scalar.dma_start(out=xTall, in_=xT_d[:, :])
        nc.gpsimd.dma_start(out=xkall, in_=xk_d[:, :])
        nc.sync.dma_start(out=wvc, in_=wvc_d[:, :])
        nc.scalar.dma_start(out=wqf, in_=wqf_d[:, :])
        nc.gpsimd.dma_start(out=wkf, in_=wkf_d[:, :])
        nc.sync.dma_start(out=pwf, in_=pwf_d[:, :])

        qT = [
            pool.tile([128, N], BF16, tag="qT", bufs=CT, name=f"qT{f}")
            for f in range(CT)
        ]
        kT = [
            pool.tile([128, NK], BF16, tag="kT", bufs=CT, name=f"kT{f}")
            for f in range(CT)
        ]
        v65 = [
            pool.tile([128, H * 65], BF16, tag="v65", bufs=NKT, name=f"v65_{i}")
            for i in range(NKT)
        ]
        aoT = [
            pool.tile([128, N], BF16, tag="aoT", bufs=CT, name=f"aoT{i}")
            for i in range(CT)
        ]

        def q_lhsT(f, c):
            if f == 0:
                return wqk0[:, c * 128 : (c + 1) * 128]
            return wqf[:, f * 768 + c * 128 : f * 768 + (c + 1) * 128]

        def k_lhsT(f, c):
            if f == 0:
                return wqk0[:, 768 + c * 128 : 768 + (c + 1) * 128]
            return wkf[:, f * 768 + c * 128 : f * 768 + (c + 1) * 128]

        def emit_q_half(f, qc):
            ps = psum.tile([128, 512], F32, tag="psS", bufs=2, name=f"ps_q{f}_{qc}")
            sl = slice(qc * 512, (qc + 1) * 512)
            for c in range(CT):
                nc.tensor.matmul(
                    ps,
                    lhsT=q_lhsT(f, c),
                    rhs=xTs(c)[:, sl],
                    start=(c == 0),
                    stop=(c == CT - 1),
                )
            nc.vector.tensor_scalar_add(
                out=qT[f][:, sl], in0=ps, scalar1=qb_sb[:, f : f + 1]
            )

        def emit_k_half(f, kc):
            w = 512 if kc == 0 else 128
            ps = psum.tile([128, w], F32, tag="psS", bufs=2, name=f"ps_k{f}_{kc}")
            sl = slice(0, 512) if kc == 0 else slice(512, 640)
            for c in range(CT):
                nc.tensor.matmul(
                    ps,
                    lhsT=k_lhsT(f, c),
                    rhs=xks(c)[:, sl],
                    start=(c == 0),
                    stop=(c == CT - 1),
                )
            nc.vector.tensor_copy(out=kT[f][:, sl], in_=ps)

        def emit_v_tile(ti):
            psa = psum.tile(
                [128, 512], F32, tag=f"psAV{ti % 2}0", bufs=1, name=f"ps_va{ti}"
            )
            psb = psum.tile(
                [128, 256], F32, tag=f"psAV{ti % 2}1", bufs=1, name=f"ps_vb{ti}"
            )
            for c in range(CT):
                nc.tensor.matmul(
                    psa,
                    lhsT=xks(c)[:, ti * 128 : (ti + 1) * 128],
                    rhs=wvc[:, c * 768 : c * 768 + 512],
                    start=(c == 0),
                    stop=(c == CT - 1),
                )
                nc.tensor.matmul(
                    psb,
                    lhsT=xks(c)[:, ti * 128 : (ti + 1) * 128],
                    rhs=wvc[:, c * 768 + 512 : (c + 1) * 768],
                    start=(c == 0),
                    stop=(c == CT - 1),
                )
            v3 = v65[ti].rearrange("p (h j) -> p h j", j=65)
            nc.vector.tensor_copy(
                out=v3[:, 0:8, 0:64], in_=psa.rearrange("p (h j) -> p h j", j=64)
            )
            nc.vector.tensor_copy(
                out=v3[:, 8:12, 0:64], in_=psb.rearrange("p (h j) -> p h j", j=64)
            )
            nc.vector.memset(v3[:, :, 64:65], 1.0)

        def emit_av_kt(pr, kt):
            for hi in range(2):
                h = pr["h0"] + hi
                for qc in range(2):
                    a = pr["aA"][kt] if qc == 0 else pr["aB"][kt]
                    nc.tensor.matmul(
                        pr["pav"][hi][qc][0:65, :],
                        lhsT=v65[kt][:, h * 65 : (h + 1) * 65],
                        rhs=a[:, hi * 512 : (hi + 1) * 512],
                        start=(kt == 0),
                        stop=(kt == NKT - 1),
                    )

        def finish_pair(pr):
            p = pr["p"]
            for hi in range(2):
                for qc in range(2):
                    j = hi * 2 + qc
                    s = pool.tile([1, 512], F32, tag=f"s{j}", bufs=2, name=f"s{p}_{j}")
                    nc.vector.tensor_copy(out=s, in_=pr["pav"][hi][qc][64:65, :])
                    r1 = pool.tile([1, 512], F32, tag=f"r1{j}", bufs=2, name=f"r1_{p}{j}")
                    nc.vector.reciprocal_approx_fast(out=r1, in_=s)
                    r2 = pool.tile([64, 512], F32, tag=f"r2{j}", bufs=2, name=f"r2_{p}{j}")
                    nc.gpsimd.partition_broadcast(r2, r1)
                    nc.vector.tensor_mul(
                        out=aoT[p][hi * 64 : (hi + 1) * 64, qc * 512 : (qc + 1) * 512],
                        in0=pr["pav"][hi][qc][0:64, :],
                        in1=r2,
                    )

        def emit_scores(p, kt, cur):
            sA = psum.tile([128, N], F32, tag="psS", bufs=2, name=f"sA{p}_{kt}")
            sB = psum.tile([128, N], F32, tag="psS", bufs=2, name=f"sB{p}_{kt}")
            for hi in range(2):
                base = hi * 64
                nc.tensor.matmul(
                    sA[:, hi * 512 : (hi + 1) * 512],
                    lhsT=kT[p][base : base + 64, kt * 128 : (kt + 1) * 128],
                    rhs=qT[p][base : base + 64, 0:512],
                    start=True,
                    stop=True,
                )
            for hi in range(2):
                base = hi * 64
                nc.tensor.matmul(
                    sB[:, hi * 512 : (hi + 1) * 512],
                    lhsT=kT[p][base : base + 64, kt * 128 : (kt + 1) * 128],
                    rhs=qT[p][base : base + 64, 512:1024],
                    start=True,
                    stop=True,
                )
            cur["sA"], cur["sB"] = sA, sB

        def emit_exps(p, kt, cur):
            aA = pool.tile([128, N], BF16, tag="attn", bufs=20, name=f"aA{p}_{kt}")
            nc.scalar.activation(aA, cur["sA"], Act.Exp, bias=mb_sb[:, kt : kt + 1])
            cur["aA"].append(aA)
            aB = pool.tile([128, N], BF16, tag="attn", bufs=20, name=f"aB{p}_{kt}")
            nc.scalar.activation(aB, cur["sB"], Act.Exp, bias=mb_sb[:, kt : kt + 1])
            cur["aB"].append(aB)

        emit_q_half(0, 0)
        emit_k_half(0, 0)
        emit_q_half(0, 1)
        emit_k_half(0, 1)  # k(0) cols 512:640 from wqk0

        prev = None
        for p in range(NPAIR):
            h0 = 2 * p
            pav = []
            for hi in range(2):
                row = [
                    psum.tile(
                        [128, 512],
                        F32,
                        tag=f"psAV{hi}{qc}",
                        bufs=1,
                        name=f"pav{h0 + hi}_{qc}",
                    )
                    for qc in range(2)
                ]
                pav.append(row)
            cur = {"p": p, "h0": h0, "aA": [], "aB": [], "pav": pav}
            for kt in range(NKT):
                emit_scores(p, kt, cur)
                if prev is not None:
                    if kt < NKT - 1:
                        emit_av_kt(prev, kt)
                        if kt == NKT - 2:
                            emit_av_kt(prev, NKT - 1)  # early drain: frees banks
                    else:
                        finish_pair(prev)  # chain completes ~phase boundary
                if p == 0:
                    emit_v_tile(kt)
                if kt == 1 and p >= 1:
                    # own-pair k cols 512:640 (needed only by scores kt==4);
                    # kept OFF the kt4 slot so the phase boundary's psS
                    # rotation turn is never extended by its eviction.
                    emit_k_half(p, 1)
                if p + 1 < NPAIR:
                    if kt == 1:
                        emit_q_half(p + 1, 0)
                    elif kt == 2:
                        emit_q_half(p + 1, 1)
                    elif kt == 3:
                        emit_k_half(p + 1, 0)
                emit_exps(p, kt, cur)
            prev = cur

        def proj_mms(ot, ps, c_list):
            for c in c_list:
                for qc in range(2):
                    nc.tensor.matmul(
                        ps[:, qc * 512 : (qc + 1) * 512],
                        lhsT=pwf[:, ot * 768 + c * 128 : ot * 768 + (c + 1) * 128],
                        rhs=aoT[c][:, qc * 512 : (qc + 1) * 512],
                        start=(c == 0),
                        stop=(c == CT - 1),
                    )

        def proj_finish(ot, ps):
            osb = pool.tile([128, N], BF16, tag="osb", bufs=3, name=f"o{ot}")
            nc.scalar.activation(osb, ps, Act.Identity, bias=pb_sb[:, ot : ot + 1])
            eng = [nc.sync, nc.scalar, nc.gpsimd][ot % 3]
            eng.dma_start(out=out_d[ot * 128 : (ot + 1) * 128, :], in_=osb)

        # drain: the last pair's AV + normalization.  ot0/ot1's c<4
        # matmuls only need aoT[0..3] (ready since phase 4), so they fill
        # the PE hole while finish(4)'s PSUM reads release the AV banks.
        ps0 = psum.tile([128, N], F32, tag="psS", bufs=2, name="ps_p0")
        proj_mms(0, ps0, range(CT - 2))
        ps1 = psum.tile([128, N], F32, tag="psS", bufs=2, name="ps_p1")
        proj_mms(1, ps1, range(CT - 2))
        for kt in range(NKT):
            emit_av_kt(prev, kt)
        finish_pair(prev)

        # ---------------- proj (continue ot0/ot1, then ot2..5) ----------
        proj_mms(0, ps0, [CT - 2])
        proj_mms(1, ps1, [CT - 2])
        proj_mms(0, ps0, [CT - 1])
        proj_finish(0, ps0)
        proj_mms(1, ps1, [CT - 1])
        proj_finish(1, ps1)
        for ot in range(2, CT):
            ps = psum.tile([128, N], F32, tag="psS", bufs=2, name=f"ps_p{ot}")
            proj_mms(ot, ps, range(CT))
            proj_finish(ot, ps)

    nc.finalize()
    return nc


def _kernel_numpy(x, mask, qkv_w, q_bias, v_bias, proj_w, proj_b):
    # exact-reference fallback (never hit for the fixed problem inputs)
    qkv_bias = np.concatenate([q_bias, np.zeros_like(v_bias), v_bias])
    out = np.empty_like(x)
    for b in range(x.shape[0]):
        qkv = x[b] @ qkv_w.T + qkv_bias
        qkv = qkv.reshape(N, 3, H, HD)
        q, k, v = (qkv[:, i].transpose(1, 0, 2) for i in range(3))
        attn = np.einsum("hqd,hkd->hqk", q * SCALE, k)
        attn = np.where(mask[b][None, None, :], -np.inf, attn)
        attn = attn - attn.max(axis=-1, keepdims=True)
        e = np.exp(attn)
        attn = e / e.sum(axis=-1, keepdims=True)
        o = np.einsum("hqk,hkd->hqd", attn, v)
        o = o.transpose(1, 0, 2).reshape(N, C)
        out[b] = o @ proj_w.T + proj_b
    return out


def _pack_weights(qkv_w, proj_w):
    wqkT = np.ascontiguousarray(qkv_w.T).astype(np.float32)  # [C, 3C]
    wqkT[:, :C] *= SCALE
    # wqf/wkf: [p, f*768 + c*128 + j] = wqkT[c*128+p, off + f*128 + j]
    wq = wqkT[:, 0:C].reshape(CT, 128, CT, 128)  # [c, p, f, j]
    wqf = np.ascontiguousarray(wq.transpose(1, 2, 0, 3).reshape(128, CT * 768))
    wk = wqkT[:, C : 2 * C].reshape(CT, 128, CT, 128)
    wkf = np.ascontiguousarray(wk.transpose(1, 2, 0, 3).reshape(128, CT * 768))
    wqk0 = np.concatenate([wqf[:, 0:768], wkf[:, 0:768]], axis=1)
    # wvc: [p, c*768 + j] = wqkT[c*128+p, 2C + j]
    wv = wqkT[:, 2 * C : 3 * C].reshape(CT, 128, C)  # [c, p, j]
    wvc = np.ascontiguousarray(wv.transpose(1, 0, 2).reshape(128, CT * 768))
    # pwf: [p, ot*768 + c*128 + j] = proj_w.T[c*128+p, ot*128+j]
    pwT = np.ascontiguousarray(proj_w.T).astype(np.float32)
    pw = pwT.reshape(CT, 128, CT, 128)
    pwf = np.ascontiguousarray(pw.transpose(1, 2, 0, 3).reshape(128, CT * 768))
    bf = ml_dtypes.bfloat16
    return wqk0.astype(bf), wqf.astype(bf), wkf.astype(bf), wvc.astype(bf), pwf.astype(bf)


def kernel(x, mask, qkv_w, q_bias, v_bias, proj_w, proj_b, **_):
    x = np.asarray(x, np.float32)
    mask = np.asarray(mask)
    qkv_w = np.asarray(qkv_w, np.float32)
    q_bias = np.asarray(q_bias, np.float32)
    v_bias = np.asarray(v_bias, np.float32)
    proj_w = np.asarray(proj_w, np.float32)
    proj_b = np.asarray(proj_b, np.float32)

    if int((~mask).sum(axis=1).max()) > NK:
        return _kernel_numpy(x, mask, qkv_w, q_bias, v_bias, proj_w, proj_b)

    wqk0, wqf, wkf, wvc, pwf = _pack_weights(qkv_w, proj_w)
    qb = (q_bias * SCALE).astype(np.float32)
    pb_eff = (proj_b + proj_w @ v_bias).astype(np.float32)

    if "nc" not in _CACHE:
        _CACHE["nc"] = _build_nc()
    nc = _CACHE["nc"]

    in_maps = []
    for b in range(B):
        idx = np.flatnonzero(~mask[b])
        nk = len(idx)
        xkT = np.zeros((C, NK), np.float32)
        xkT[:, :nk] = x[b][idx].T
        mb = np.full(NK, MASK_NEG, np.float32)
        mb[:nk] = 0.0
        xTr = (
            np.ascontiguousarray(x[b].T)
            .reshape(CT, 128, N)
            .transpose(1, 0, 2)
            .reshape(128, CT * N)
        )
        xkr = xkT.reshape(CT, 128, NK).transpose(1, 0, 2).reshape(128, CT * NK)
        in_maps.append(
            {
                "xTr": np.ascontiguousarray(xTr).astype(ml_dtypes.bfloat16),
                "xkr": np.ascontiguousarray(xkr).astype(ml_dtypes.bfloat16),
                "wqk0": wqk0,
                "wqf": wqf,
                "wkf": wkf,
                "wvc": wvc,
                "pwf": pwf,
                "qb": qb,
                "mb": mb,
                "pb": pb_eff,
            }
        )

    _CACHE["last_in_maps"] = in_maps
    res = run_bass_kernel_spmd(nc, in_maps, list(range(B)))
    out = np.stack(
        [res.results[b]["out"].astype(np.float32).T for b in range(B)], axis=0
    )
    return np.ascontiguousarray(out)


if __name__ == "__main__":
    np.random.seed(0)
    x = np.random.randn(B, N, C).astype(np.float32)
    mask = np.random.randint(0, 2, (B, N)) > 0
    qkv_w = (np.random.randn(F3, C) * 0.02).astype(np.float32)
    q_bias = (np.random.randn(C) * 0.02).astype(np.float32)
    v_bias = (np.random.randn(C) * 0.02).astype(np.float32)
    proj_w = (np.random.randn(C, C) * 0.02).astype(np.float32)
    proj_b = (np.random.randn(C) * 0.02).astype(np.float32)
    out = kernel(x, mask, qkv_w, q_bias, v_bias, proj_w, proj_b)
    ref = _kernel_numpy(x, mask, qkv_w, q_bias, v_bias, proj_w, proj_b)
    rel = np.linalg.norm(out - ref) / np.linalg.norm(ref)
    print(out.shape, out.dtype, "rel err vs numpy:", rel)


# revision 36
# speedup vs baseline: 1.0210x; 1.0210x over previous
"""Trainium2 Bass kernel for AttentionWithCAE.

Reference computation (B=8, N=1024, C=768, H=12, hd=64):
    qkv  = x @ qkv_w.T + concat(q_bias, 0, v_bias)
    q,k,v per head; attn = softmax(mask(q*scale @ k.T)); out = attn @ v
    final = out @ proj_w.T + proj_b

Sharding: pure data parallel -- batch b on core b, weights replicated,
no collectives.

Structural ideas (in rough order of impact):

1. Key gather: the mask removes ~50% of the KEYS (True = masked out;
   queries are never masked).  The host gathers only the unmasked key
   tokens per batch and pads to NK=640 (max real count is 530), so
   scores, exp, attn@v and the k/v projections shrink by 0.625.  Exact:
   softmax is permutation-invariant over keys; padding keys have zeroed
   x columns and a -30000 exp bias so they contribute exactly 0.

2. Scores transposed [k, q] with the two heads of a pair side by side in
   one [128, 1024] PSUM tile per q-chunk (tileA = q0:512, tileB =
   q512:1024; head hi in columns hi*512..).  The two K=64 matmuls
   filling a tile target row groups 0:64 / 64:128 and are emitted
   back-to-back, so they run CONCURRENTLY on the PE array (measured
   dt ~5ns).  One exp ACTIVATE per tile covers both heads (mask bias is
   per-partition = per key, identical for both).

3. ACT (exp) paces the attention phases.  Pair p's attn@v (with the
   baked ones-column giving the softmax denominators as PSUM row 64) is
   DEFERRED into phase p+1 and interleaved kt-by-kt so its matmuls never
   wait on fresh exps; the last AV chunk is pulled one slot early
   (slot 3) so the normalization chain (slot 4) completes by the phase
   boundary and the next pair's AV never stalls on the PSUM banks.

4. Normalization: denominator row -> partition 0 (single-partition
   custom/ISA ops only work at base 0), reciprocal_approx_fast on DVE,
   partition_broadcast on the otherwise-idle GPSIMD, then one DVE
   multiply per (head, q-chunk) STRAIGHT from the AV PSUM into aoT.

5. DMA: descriptor rows stripe across all 16 DMA engines, so few FAT
   DMAs beat many small ones; dispatch costs ~0.7us per dma_start on an
   engine queue, so inputs are host-packed into 8 large tensors (rows
   1.5-12KB), issued round-robin over the sync/scalar/gpsimd queues in
   earliest-need order.  Output is bf16 (host upcasts).

6. HAM warm-up: ~4us of F=512 garbage matmuls during the DMA dead time
   so the PE is at full clock when real work arrives (F=8 matmuls do
   NOT warm it -- only ~5% array duty).

7. Tail: proj is ot-major from the scores PSUM slots; ot0/ot1's c<5
   matmuls are emitted before the c==5 ones so the last pair's
   normalization latency is fully hidden behind real work.
"""

import sys

sys.path.insert(0, "/opt/trn_rl_repo")

from contextlib import ExitStack

import numpy as np
import ml_dtypes

import concourse.bass as bass
import concourse.bacc as bacc
import concourse.mybir as mybir
from concourse import tile
from concourse.bass_utils import run_bass_kernel_spmd

B, N, C = 8, 1024, 768
H, HD = 12, 64
F3 = 3 * C  # 2304
SCALE = HD ** -0.5
F32 = mybir.dt.float32
BF16 = mybir.dt.bfloat16
Act = mybir.ActivationFunctionType

MASK_NEG = -30000.0

NK = 640  # padded unmasked-key count (max real count is ~530)
NKT = NK // 128  # 5 key tiles
CT = C // 128  # 6 contraction tiles
NPAIR = H // 2  # 6 head pairs

_CACHE = {}


def _build_nc():
    nc = bacc.Bacc(None, target_bir_lowering=False)

    # host-packed layouts (see _pack_weights / kernel()):
    #   xTr  [128, CT*1024] queries, c-major:   [p, c*1024 + t]
    #   xkr  [128, CT*640]  gathered keys, c-major
    #   wqk0 [128, 1536]    q/k weights feature-tile 0: [wq_f0 | wk_f0]
    #   wqf  [128, CT*768]  q weights, feature-tile-major
    #   wkf  [128, CT*768]  k weights, feature-tile-major
    #   wvc  [128, CT*768]  v weights, c-major
    #   pwf  [128, CT*768]  proj weights, out-tile-major
    xT_d = nc.declare_dram_parameter("xTr", [128, CT * 1024], BF16, isOutput=False)
    xk_d = nc.declare_dram_parameter("xkr", [128, CT * 640], BF16, isOutput=False)
    wqk0_d = nc.declare_dram_parameter("wqk0", [128, 1536], BF16, isOutput=False)
    wqf_d = nc.declare_dram_parameter("wqf", [128, CT * 768], BF16, isOutput=False)
    wkf_d = nc.declare_dram_parameter("wkf", [128, CT * 768], BF16, isOutput=False)
    wvc_d = nc.declare_dram_parameter("wvc", [128, CT * 768], BF16, isOutput=False)
    pwf_d = nc.declare_dram_parameter("pwf", [128, CT * 768], BF16, isOutput=False)
    qb_d = nc.declare_dram_parameter("qb", [C], F32, isOutput=False)
    mb_d = nc.declare_dram_parameter("mb", [NK], F32, isOutput=False)
    pb_d = nc.declare_dram_parameter("pb", [C], F32, isOutput=False)
    out_d = nc.declare_dram_parameter("out", [C, N], BF16, isOutput=True)

    with ExitStack() as ctx:
        tc = ctx.enter_context(tile.TileContext(nc))
        pool = ctx.enter_context(tc.tile_pool(name="main", bufs=1))
        psum = ctx.enter_context(tc.tile_pool(name="psum", bufs=1, space="PSUM"))

        # preload the exp table set while DMAs run (no data dependency)
        tiny = pool.tile([1, 8], F32)
        nc.vector.memset(tiny, 0.0)
        tiny2 = pool.tile([1, 8], F32)
        nc.scalar.activation(tiny2, tiny, Act.Exp)

        qb_sb = pool.tile([128, CT], F32)
        nc.sync.dma_start(out=qb_sb, in_=qb_d.rearrange("(i p) -> p i", p=128))
        mb_sb = pool.tile([128, NKT], F32)
        nc.scalar.dma_start(out=mb_sb, in_=mb_d.rearrange("(i p) -> p i", p=128))
        pb_sb = pool.tile([128, CT], F32)
        nc.gpsimd.dma_start(out=pb_sb, in_=pb_d.rearrange("(i p) -> p i", p=128))

        # HAM warm-up: full-width F=512 matmuls (high array duty) bridging
        # the whole DMA wait so real work starts at 2.4GHz.
        warm_w = pool.tile([128, 128], BF16, name="warmw")
        nc.vector.memset(warm_w, 0.25)
        warm_x = pool.tile([128, 512], BF16, name="warmx")
        nc.vector.memset(warm_x, 0.25)
        wps = psum.tile([128, 512], F32, tag="psAV00", bufs=1, name="warm")
        for wi in range(80):
            nc.tensor.matmul(wps, lhsT=warm_w, rhs=warm_x, start=True, stop=True)

        xTall = pool.tile([128, CT * 1024], BF16, name="xTall")
        xkall = pool.tile([128, CT * 640], BF16, name="xkall")
        wqk0 = pool.tile([128, 1536], BF16, name="wqk0")
        wvc = pool.tile([128, CT * 768], BF16, name="wvc")
        wqf = pool.tile([128, CT * 768], BF16, name="wqf")
        wkf = pool.tile([128, CT * 768], BF16, name="wkf")
        pwf = pool.tile([128, CT * 768], BF16, name="pwf")

        def xTs(c):
            return xTall[:, c * 1024 : (c + 1) * 1024]

        def xks(c):
            return xkall[:, c * 640 : (c + 1) * 640]

        # input DMAs: few fat transfers, earliest-need first, spread over
        # three dispatch queues.
        nc.sync.dma_start(out=wqk0, in_=wqk0_d[:, :])
        nc.scalar.dma_start(out=xTall, in_=xT_d[:, :])
        nc.gpsimd.dma_start(out=xkall, in_=xk_d[:, :])
        nc.sync.dma_start(out=wvc, in_=wvc_d[:, :])
        nc.scalar.dma_start(out=wqf, in_=wqf_d[:, :])
        nc.gpsimd.dma_start(out=wkf, in_=wkf_d[:, :])
        nc.sync.dma_start(out=pwf, in_=pwf_d[:, :])

        qT = [
            pool.tile([128, N], BF16, tag="qT", bufs=CT, name=f"qT{f}")
            for f in range(CT)
        ]
        kT = [
            pool.tile([128, NK], BF16, tag="kT", bufs=CT, name=f"kT{f}")
            for f in range(CT)
        ]
        v65 = [
            pool.tile([128, H * 65], BF16, tag="v65", bufs=NKT, name=f"v65_{i}")
            for i in range(NKT)
        ]
        aoT = [
            pool.tile([128, N], BF16, tag="aoT", bufs=CT, name=f"aoT{i}")
            for i in range(CT)
        ]

        def q_lhsT(f, c):
            if f == 0:
                return wqk0[:, c * 128 : (c + 1) * 128]
            return wqf[:, f * 768 + c * 128 : f * 768 + (c + 1) * 128]

        def k_lhsT(f, c):
            if f == 0:
                return wqk0[:, 768 + c * 128 : 768 + (c + 1) * 128]
            return wkf[:, f * 768 + c * 128 : f * 768 + (c + 1) * 128]

        def emit_q_half(f, qc):
            ps = psum.tile([128, 512], F32, tag="psS", bufs=2, name=f"ps_q{f}_{qc}")
            sl = slice(qc * 512, (qc + 1) * 512)
            for c in range(CT):
                nc.tensor.matmul(
                    ps,
                    lhsT=q_lhsT(f, c),
                    rhs=xTs(c)[:, sl],
                    start=(c == 0),
                    stop=(c == CT - 1),
                )
            nc.vector.tensor_scalar_add(
                out=qT[f][:, sl], in0=ps, scalar1=qb_sb[:, f : f + 1]
            )

        def emit_k_half(f, kc):
            w = 512 if kc == 0 else 128
            ps = psum.tile([128, w], F32, tag="psS", bufs=2, name=f"ps_k{f}_{kc}")
            sl = slice(0, 512) if kc == 0 else slice(512, 640)
            for c in range(CT):
                nc.tensor.matmul(
                    ps,
                    lhsT=k_lhsT(f, c),
                    rhs=xks(c)[:, sl],
                    start=(c == 0),
                    stop=(c == CT - 1),
                )
            nc.vector.tensor_copy(out=kT[f][:, sl], in_=ps)

        def emit_v_tile(ti):
            psa = psum.tile(
                [128, 512], F32, tag=f"psAV{ti % 2}0", bufs=1, name=f"ps_va{ti}"
            )
            psb = psum.tile(
                [128, 256], F32, tag=f"psAV{ti % 2}1", bufs=1, name=f"ps_vb{ti}"
            )
            for c in range(CT):
                nc.tensor.matmul(
                    psa,
                    lhsT=xks(c)[:, ti * 128 : (ti + 1) * 128],
                    rhs=wvc[:, c * 768 : c * 768 + 512],
                    start=(c == 0),
                    stop=(c == CT - 1),
                )
                nc.tensor.matmul(
                    psb,
                    lhsT=xks(c)[:, ti * 128 : (ti + 1) * 128],
                    rhs=wvc[:, c * 768 + 512 : (c + 1) * 768],
                    start=(c == 0),
                    stop=(c == CT - 1),
                )
            v3 = v65[ti].rearrange("p (h j) -> p h j", j=65)
            nc.vector.tensor_copy(
                out=v3[:, 0:8, 0:64], in_=psa.rearrange("p (h j) -> p h j", j=64)
            )
            nc.vector.tensor_copy(
                out=v3[:, 8:12, 0:64], in_=psb.rearrange("p (h j) -> p h j", j=64)
            )
            nc.vector.memset(v3[:, :, 64:65], 1.0)

        def emit_av_kt(pr, kt):
            for hi in range(2):
                h = pr["h0"] + hi
                for qc in range(2):
                    a = pr["aA"][kt] if qc == 0 else pr["aB"][kt]
                    nc.tensor.matmul(
                        pr["pav"][hi][qc][0:65, :],
                        lhsT=v65[kt][:, h * 65 : (h + 1) * 65],
                        rhs=a[:, hi * 512 : (hi + 1) * 512],
                        start=(kt == 0),
                        stop=(kt == NKT - 1),
                    )

        def finish_pair(pr):
            p = pr["p"]
            for hi in range(2):
                for qc in range(2):
                    j = hi * 2 + qc
                    s = pool.tile([1, 512], F32, tag=f"s{j}", bufs=2, name=f"s{p}_{j}")
                    nc.vector.tensor_copy(out=s, in_=pr["pav"][hi][qc][64:65, :])
                    r1 = pool.tile([1, 512], F32, tag=f"r1{j}", bufs=2, name=f"r1_{p}{j}")
                    nc.vector.reciprocal_approx_fast(out=r1, in_=s)
                    r2 = pool.tile([64, 512], F32, tag=f"r2{j}", bufs=2, name=f"r2_{p}{j}")
                    nc.gpsimd.partition_broadcast(r2, r1)
                    nc.vector.tensor_mul(
                        out=aoT[p][hi * 64 : (hi + 1) * 64, qc * 512 : (qc + 1) * 512],
                        in0=pr["pav"][hi][qc][0:64, :],
                        in1=r2,
                    )

        def emit_scores(p, kt, cur):
            sA = psum.tile([128, N], F32, tag="psS", bufs=2, name=f"sA{p}_{kt}")
            sB = psum.tile([128, N], F32, tag="psS", bufs=2, name=f"sB{p}_{kt}")
            for hi in range(2):
                base = hi * 64
                nc.tensor.matmul(
                    sA[:, hi * 512 : (hi + 1) * 512],
                    lhsT=kT[p][base : base + 64, kt * 128 : (kt + 1) * 128],
                    rhs=qT[p][base : base + 64, 0:512],
                    start=True,
                    stop=True,
                )
            for hi in range(2):
                base = hi * 64
                nc.tensor.matmul(
                    sB[:, hi * 512 : (hi + 1) * 512],
                    lhsT=kT[p][base : base + 64, kt * 128 : (kt + 1) * 128],
                    rhs=qT[p][base : base + 64, 512:1024],
                    start=True,
                    stop=True,
                )
            cur["sA"], cur["sB"] = sA, sB

        def emit_exps(p, kt, cur):
            aA = pool.tile([128, N], BF16, tag="attn", bufs=20, name=f"aA{p}_{kt}")
            nc.scalar.activation(aA, cur["sA"], Act.Exp, bias=mb_sb[:, kt : kt + 1])
            cur["aA"].append(aA)
            aB = pool.tile([128, N], BF16, tag="attn", bufs=20, name=f"aB{p}_{kt}")
            nc.scalar.activation(aB, cur["sB"], Act.Exp, bias=mb_sb[:, kt : kt + 1])
            cur["aB"].append(aB)

        emit_q_half(0, 0)
        emit_k_half(0, 0)
        emit_q_half(0, 1)
        emit_k_half(0, 1)  # k(0) cols 512:640 from wqk0

        prev = None
        for p in range(NPAIR):
            h0 = 2 * p
            pav = []
            for hi in range(2):
                row = [
                    psum.tile(
                        [128, 512],
                        F32,
                        tag=f"psAV{hi}{qc}",
                        bufs=1,
                        name=f"pav{h0 + hi}_{qc}",
                    )
                    for qc in range(2)
                ]
                pav.append(row)
            cur = {"p": p, "h0": h0, "aA": [], "aB": [], "pav": pav}
            for kt in range(NKT):
                emit_scores(p, kt, cur)
                if prev is not None:
                    if kt < NKT - 1:
                        emit_av_kt(prev, kt)
                        if kt == NKT - 2:
                            emit_av_kt(prev, NKT - 1)  # early drain: frees banks
                    else:
                        finish_pair(prev)  # chain completes ~phase boundary
                if p == 0:
                    emit_v_tile(kt)
                if p + 1 < NPAIR:
                    if kt == 1:
                        emit_q_half(p + 1, 0)
                    elif kt == 2:
                        emit_q_half(p + 1, 1)
                    elif kt == 3:
                        emit_k_half(p + 1, 0)
                    elif kt == 4:
                        emit_k_half(p + 1, 1)
                emit_exps(p, kt, cur)
            prev = cur

        def proj_mms(ot, ps, c_list):
            for c in c_list:
                for qc in range(2):
                    nc.tensor.matmul(
                        ps[:, qc * 512 : (qc + 1) * 512],
                        lhsT=pwf[:, ot * 768 + c * 128 : ot * 768 + (c + 1) * 128],
                        rhs=aoT[c][:, qc * 512 : (qc + 1) * 512],
                        start=(c == 0),
                        stop=(c == CT - 1),
                    )

        def proj_finish(ot, ps):
            osb = pool.tile([128, N], BF16, tag="osb", bufs=3, name=f"o{ot}")
            nc.scalar.activation(osb, ps, Act.Identity, bias=pb_sb[:, ot : ot + 1])
            eng = [nc.sync, nc.scalar, nc.gpsimd][ot % 3]
            eng.dma_start(out=out_d[ot * 128 : (ot + 1) * 128, :], in_=osb)

        # drain: the last pair's AV + normalization.  ot0/ot1's c<4
        # matmuls only need aoT[0..3] (ready since phase 4), so they fill
        # the PE hole while finish(4)'s PSUM reads release the AV banks.
        ps0 = psum.tile([128, N], F32, tag="psS", bufs=2, name="ps_p0")
        proj_mms(0, ps0, range(CT - 2))
        ps1 = psum.tile([128, N], F32, tag="psS", bufs=2, name="ps_p1")
        proj_mms(1, ps1, range(CT - 2))
        for kt in range(NKT):
            emit_av_kt(prev, kt)
        finish_pair(prev)

        # ---------------- proj (continue ot0/ot1, then ot2..5) ----------
        proj_mms(0, ps0, [CT - 2])
        proj_mms(1, ps1, [CT - 2])
        proj_mms(0, ps0, [CT - 1])
        proj_finish(0, ps0)
        proj_mms(1, ps1, [CT - 1])
        proj_finish(1, ps1)
        for ot in range(2, CT):
            ps = psum.tile([128, N], F32, tag="psS", bufs=2, name=f"ps_p{ot}")
            proj_mms(ot, ps, range(CT))
            proj_finish(ot, ps)

    nc.finalize()
    return nc


def _kernel_numpy(x, mask, qkv_w, q_bias, v_bias, proj_w, proj_b):
    # exact-reference fallback (never hit for the fixed problem inputs)
    qkv_bias = np.concatenate([q_bias, np.zeros_like(v_bias), v_bias])
    out = np.empty_like(x)
    for b in range(x.shape[0]):
        qkv = x[b] @ qkv_w.T + qkv_bias
        qkv = qkv.reshape(N, 3, H, HD)
        q, k, v = (qkv[:, i].transpose(1, 0, 2) for i in range(3))
        attn = np.einsum("hqd,hkd->hqk", q * SCALE, k)
        attn = np.where(mask[b][None, None, :], -np.inf, attn)
        attn = attn - attn.max(axis=-1, keepdims=True)
        e = np.exp(attn)
        attn = e / e.sum(axis=-1, keepdims=True)
        o = np.einsum("hqk,hkd->hqd", attn, v)
        o = o.transpose(1, 0, 2).reshape(N, C)
        out[b] = o @ proj_w.T + proj_b
    return out


def _pack_weights(qkv_w, proj_w):
    wqkT = np.ascontiguousarray(qkv_w.T).astype(np.float32)  # [C, 3C]
    wqkT[:, :C] *= SCALE
    # wqf/wkf: [p, f*768 + c*128 + j] = wqkT[c*128+p, off + f*128 + j]
    wq = wqkT[:, 0:C].reshape(CT, 128, CT, 128)  # [c, p, f, j]
    wqf = np.ascontiguousarray(wq.transpose(1, 2, 0, 3).reshape(128, CT * 768))
    wk = wqkT[:, C : 2 * C].reshape(CT, 128, CT, 128)
    wkf = np.ascontiguousarray(wk.transpose(1, 2, 0, 3).reshape(128, CT * 768))
    wqk0 = np.concatenate([wqf[:, 0:768], wkf[:, 0:768]], axis=1)
    # wvc: [p, c*768 + j] = wqkT[c*128+p, 2C + j]
    wv = wqkT[:, 2 * C : 3 * C].reshape(CT, 128, C)  # [c, p, j]
    wvc = np.ascontiguousarray(wv.transpose(1, 0, 2).reshape(128, CT * 768))
    # pwf: [p, ot*768 + c*128 + j] = proj_w.T[c*128+p, ot*128+j]
    pwT = np.ascontiguousarray(proj_w.T).astype(np.float32)
    pw = pwT.reshape(CT, 128, CT, 128)
    pwf = np.ascontiguousarray(pw.transpose(1, 2, 0, 3).reshape(128, CT * 768))
    bf = ml_dtypes.bfloat16
    return wqk0.astype(bf), wqf.astype(bf), wkf.astype(bf), wvc.astype(bf), pwf.astype(bf)


def kernel(x, mask, qkv_w, q_bias, v_bias, proj_w, proj_b, **_):
    x = np.asarray(x, np.float32)
    mask = np.asarray(mask)
    qkv_w = np.asarray(qkv_w, np.float32)
    q_bias = np.asarray(q_bias, np.float32)
    v_bias = np.asarray(v_bias, np.float32)
    proj_w = np.asarray(proj_w, np.float32)
    proj_b = np.asarray(proj_b, np.float32)

    if int((~mask).sum(axis=1).max()) > NK:
        return _kernel_numpy(x, mask, qkv_w, q_bias, v_bias, proj_w, proj_b)

    wqk0, wqf, wkf, wvc, pwf = _pack_weights(qkv_w, proj_w)
    qb = (q_bias * SCALE).astype(np.float32)
    pb_eff = (proj_b + proj_w @ v_bias).astype(np.float32)

    if "nc" not in _CACHE:
        _CACHE["nc"] = _build_nc()
    nc = _CACHE["nc"]

    in_maps = []
    for b in range(B):
        idx = np.flatnonzero(~mask[b])
        nk = len(idx)
        xkT = np.zeros((C, NK), np.float32)
        xkT[:, :nk] = x[b][idx].T
        mb = np.full(NK, MASK_NEG, np.float32)
        mb[:nk] = 0.0
        xTr = (
            np.ascontiguousarray(x[b].T)
            .reshape(CT, 128, N)
            .transpose(1, 0, 2)
            .reshape(128, CT * N)
        )
        xkr = xkT.reshape(CT, 128, NK).transpose(1, 0, 2).reshape(128, CT * NK)
        in_maps.append(
            {
                "xTr": np.ascontiguousarray(xTr).astype(ml_dtypes.bfloat16),
                "xkr": np.ascontiguousarray(xkr).astype(ml_dtypes.bfloat16),
                "wqk0": wqk0,
                "wqf": wqf,
                "wkf": wkf,
                "wvc": wvc,
                "pwf": pwf,
                "qb": qb,
                "mb": mb,
                "pb": pb_eff,
            }
        )

    _CACHE["last_in_maps"] = in_maps
    res = run_bass_kernel_spmd(nc, in_maps, list(range(B)))
    out = np.stack(
        [res.results[b]["out"].astype(np.float32).T for b in range(B)], axis=0
    )
    return np.ascontiguousarray(out)


if __name__ == "__main__":
    np.random.seed(0)
    x = np.random.randn(B, N, C).astype(np.float32)
    mask = np.random.randint(0, 2, (B, N)) > 0
    qkv_w = (np.random.randn(F3, C) * 0.02).astype(np.float32)
    q_bias = (np.random.randn(C) * 0.02).astype(np.float32)
    v_bias = (np.random.randn(C) * 0.02).astype(np.float32)
    proj_w = (np.random.randn(C, C) * 0.02).astype(np.float32)
    proj_b = (np.random.randn(C) * 0.02).astype(np.float32)
    out = kernel(x, mask, qkv_w, q_bias, v_bias, proj_w, proj_b)
    ref = _kernel_numpy(x, mask, qkv_w, q_bias, v_bias, proj_w, proj_b)
    rel = np.linalg.norm(out - ref) / np.linalg.norm(ref)
    print(out.shape, out.dtype, "rel err vs numpy:", rel)
